# revision 1
# baseline (speedup 1.0000x reference)
"""DifferentialAttention TRN2 Bass kernel — 8-core SPMD.

Sharding: core c handles batch b = c//4, query rows [512*(c%4), 512*(c%4+1)).
Each core computes full K/V for its batch (replicated across the 4 cores of
that batch), its 512-query slice of Q, attention, subnorm, and the full
out-projection for its query slice. No collectives needed; host concatenates.

Per-core dataflow (feature-major):
  x_b --DMA--> stage --PE transpose--> xT (e-major)  [256-position groups]
  K^T = Wk'.T @ xT ; V = xT.T @ Wv ; Q^T = Wq'.T @ xT[:, qslice]
  RoPE (de-interleaved layout; pair-swap via SBUF-SBUF DMA; 3 DVE ops)
  scores^T[k,q] per comp pair via row-group-paired matmuls (K=64 x2)
  E = exp(scores^T) on ACT (psum->sbuf, fp32r out)
  U_i = E_i.T-contracted with V via PE (k-major accumulate over 16 blocks)
  s_i = ones.T @ E_i (PE), 1/s via DVE, broadcast via K=1 matmul
  O = U1/s1 - lam*U2/s2 ; RMS over features via ones-matmul + sqrt + bcast
  y^T = Wo'.T @ O^T accumulated over heads; DMA y^T (1024,512) out.

Weights are host-permuted: de-interleaved RoPE pairs (so the pair-swap is a
32-partition block swap), attention scale folded into Wq, subnorm weight
folded into Wo rows.
"""
import sys

sys.path.insert(0, '/opt/trn_rl_repo')

import math
import time
from contextlib import ExitStack

import numpy as np

B, S, E = 2, 2048, 1024
NH, HD = 8, 64
SQ = 512            # queries per core
NCORES = 8
LAMBDA_INIT = 0.8
EPS = 1e-5

_CACHE = {}


def _build(reps=None, phase_limit=3):
    import concourse.bacc as bacc
    import concourse.tile as tile
    from concourse import mybir
    from concourse.masks import make_identity

    dt = mybir.dt
    F32, F32R = dt.float32, dt.float32r
    AF = mybir.ActivationFunctionType

    nc = bacc.Bacc("TRN2", num_devices=NCORES)
    d_x = nc.declare_dram_parameter("xb", [S, E], F32, isOutput=False)
    d_wq = nc.declare_dram_parameter("wq", [E, E], F32R, isOutput=False)
    d_wk = nc.declare_dram_parameter("wk", [E, 256], F32R, isOutput=False)
    d_wv = nc.declare_dram_parameter("wv", [E, 256], F32R, isOutput=False)
    d_wo = nc.declare_dram_parameter("wo", [E, E], F32R, isOutput=False)
    d_kcos = nc.declare_dram_parameter("kcos", [128, S], F32, isOutput=False)
    d_ksin = nc.declare_dram_parameter("ksin", [128, S], F32, isOutput=False)
    d_qcos = nc.declare_dram_parameter("qcos", [128, SQ], F32, isOutput=False)
    d_qsin = nc.declare_dram_parameter("qsin", [128, SQ], F32, isOutput=False)
    d_ones128 = nc.declare_dram_parameter("ones128", [128, 1], F32R,
                                          isOutput=False)
    d_ones1x = nc.declare_dram_parameter("ones1x", [1, 128], F32R,
                                         isOutput=False)
    d_lam1x = nc.declare_dram_parameter("lam1x", [1, 128], F32R,
                                        isOutput=False)
    d_pswap = nc.declare_dram_parameter("pswap", [128, 128], F32R,
                                        isOutput=False)
    d_y = nc.declare_dram_parameter("yT", [E, SQ], F32, isOutput=True)
    d_scr = [[nc.dram_tensor(f"scr{i}_{h}", [1, 512], mybir.dt.float32)
              for i in range(3)] for h in range(NH)]

    with tile.TileContext(nc) as tc, ExitStack() as ctx:
        sb1 = ctx.enter_context(tc.tile_pool(name="sb1", bufs=1))
        sbstage = ctx.enter_context(tc.tile_pool(name="sbstage", bufs=2))
        sbxt2 = ctx.enter_context(tc.tile_pool(name="sbxt2", bufs=4))
        sbxtq = ctx.enter_context(tc.tile_pool(name="sbxtq", bufs=8))
        sbw = ctx.enter_context(tc.tile_pool(name="sbw", bufs=2))
        sbkv = ctx.enter_context(tc.tile_pool(name="sbkv", bufs=8))
        sbv = ctx.enter_context(tc.tile_pool(name="sbv", bufs=16))
        sbe = ctx.enter_context(tc.tile_pool(name="sbe", bufs=5))
        sbo = ctx.enter_context(tc.tile_pool(name="sbo", bufs=8))
        sbtmp = ctx.enter_context(tc.tile_pool(name="sbtmp", bufs=2))
        smalls = ctx.enter_context(tc.tile_pool(name="smalls", bufs=1))
        psP = ctx.enter_context(tc.tile_pool(name="psP", bufs=2, space="PSUM"))
        psQ = ctx.enter_context(tc.tile_pool(name="psQ", bufs=1, space="PSUM"))

        def _emit():
            # ---- persistent tiles
            krot = sb1.tile([128, 2 * S], F32R, tag="krot")
            qrot = sb1.tile([128, 8 * SQ], F32R, tag="qrot")
            kcos = sb1.tile([128, S], F32, tag="kcos")
            ksin = sb1.tile([128, S], F32, tag="ksin")
            qcos = sb1.tile([128, SQ], F32, tag="qcos")
            qsin = sb1.tile([128, SQ], F32, tag="qsin")
            ident = sb1.tile([128, 128], F32, tag="ident")
            ones128 = sb1.tile([128, 1], F32R, tag="ones128")
            ones1x = sb1.tile([1, 128], F32R, tag="ones1x")
            lam1x = sb1.tile([1, 128], F32R, tag="lam1x")
            nc.sync.dma_start(ones1x[:], d_ones1x[:])
            nc.sync.dma_start(lam1x[:], d_lam1x[:])
            lamsc = sb1.tile([1, 1], F32, tag="lamsc")
            nc.sync.dma_start(lamsc[:], d_lam1x[0:1, 0:1].bitcast(F32))
            epsb = sb1.tile([1, 1], F32, tag="epsb")
            nc.vector.memset(epsb[:], EPS)
            zerob = sb1.tile([1, 1], F32, tag="zerob")
            nc.vector.memset(zerob[:], 0.0)
            pswap = sb1.tile([128, 128], F32R, tag="pswap")
            nc.sync.dma_start(pswap[:], d_pswap[:])

            nc.sync.dma_start(kcos[:], d_kcos[:])
            nc.sync.dma_start(ksin[:], d_ksin[:])
            nc.sync.dma_start(qcos[:], d_qcos[:])
            nc.sync.dma_start(qsin[:], d_qsin[:])
            nc.sync.dma_start(ones128[:], d_ones128[:])
            make_identity(nc, ident[:])

            # wk/wv resident in full (8 e-chunks each)
            wk_t = [sbkv.tile([128, 256], F32R, tag="wk", name=f"wk{e}")
                    for e in range(8)]
            wv_t = [sbkv.tile([128, 256], F32R, tag="wv", name=f"wv{e}")
                    for e in range(8)]
            for e in range(8):
                nc.sync.dma_start(wk_t[e][:], d_wk[e * 128:(e + 1) * 128, :])
                nc.sync.dma_start(wv_t[e][:], d_wv[e * 128:(e + 1) * 128, :])

            v_sb = [sbv.tile([128, 256], F32R, tag="v", name=f"v{p}")
                    for p in range(16)]
            xtq = [sbxtq.tile([128, SQ], F32R, tag="xtq", name=f"xtq{e}")
                   for e in range(8)]

            # ---- phase 0+1: transpose x by 256-position groups; K^T, V proj
            # One NEFF serves all 8 cores: the host pre-rotates x along S so
            # every core's q-slice sits at rows [0, 512). Jointly permuting
            # keys and values leaves softmax attention invariant; the K rope
            # tables are rotated identically so K keeps original positions.

            wq_r = d_wq.ap().rearrange("(c p) f -> p c f", p=128)

            def qpass(fb):
                wq_c = sbw.tile([128, 1024], F32R, tag="wcol",
                                name=f"wq{fb}")
                nc.sync.dma_start(
                    wq_c[:].rearrange("p (c f) -> p c f", c=8),
                    wq_r[:, :, fb * 128:(fb + 1) * 128])
                pq = psQ.tile([128, 512], F32, tag="u1", name=f"pq{fb}")
                for e in range(8):
                    nc.tensor.matmul(pq[:], wq_c[:, e * 128:(e + 1) * 128],
                                     xtq[e][:],
                                     start=(e == 0), stop=(e == 7))
                pl = sbtmp.tile([128, 512], F32R, tag="plain",
                                name=f"qpl{fb}")
                nc.scalar.copy(pl[:], pq[:])
                psw = psQ.tile([128, 512], F32, tag="s1", name=f"qsw{fb}")
                nc.tensor.matmul(psw[:], pswap[:], pl[:],
                                 start=True, stop=True)
                t1 = sbtmp.tile([128, 512], F32, tag="ropeA",
                                name=f"qt1_{fb}")
                nc.vector.tensor_mul(t1[:], pq[:], qcos[:])
                t2 = sbtmp.tile([128, 512], F32, tag="ropeB",
                                name=f"qt2_{fb}")
                nc.vector.tensor_mul(t2[:], psw[:], qsin[:])
                nc.vector.tensor_add(qrot[:, fb * SQ:(fb + 1) * SQ],
                                     t1[:], t2[:])

            for pb2 in range(8):
                stg = sbstage.tile([128, 2048], F32, tag="stage")
                for h2 in range(2):
                    pb = pb2 * 2 + h2
                    nc.sync.dma_start(stg[:, h2 * 1024:(h2 + 1) * 1024],
                                      d_x[pb * 128:(pb + 1) * 128, :])
                xt4 = [sbxt2.tile([128, 1024], F32R, tag="xt4",
                                   name=f"xt4_{pb2}_{eh}") for eh in range(2)]

                def _xt(e):
                    # e-chunk e of this group: columns [ (e%4)*256, +256 )
                    return xt4[e // 4][:, (e % 4) * 256:(e % 4) * 256 + 256]

                for eh in range(2):  # split 8 e-chunks over 2 psum groups
                    pt = psP.tile([128, 1024], F32, tag="sp")
                    for j in range(4):
                        e = eh * 4 + j
                        for h2 in range(2):
                            nc.tensor.transpose(
                                pt[:, j * 256 + h2 * 128:j * 256 + h2 * 128 + 128],
                                stg[:, h2 * 1024 + e * 128:h2 * 1024 + e * 128 + 128],
                                ident[:])
                    nc.scalar.copy(xt4[eh][:], pt[:])
                # V for the two position blocks of this group
                for h2 in range(2):
                    pb = pb2 * 2 + h2
                    pv = psQ.tile([128, 256], F32, tag="u1")
                    for e in range(8):
                        xe = _xt(e)
                        nc.tensor.matmul(pv[:], xe[:, h2 * 128:h2 * 128 + 128],
                                         wv_t[e][:],
                                         start=(e == 0), stop=(e == 7))
                    nc.scalar.copy(v_sb[pb][:], pv[:])
                # K^T columns for this group, + rope
                for fb in range(2):
                    pk = psQ.tile([128, 256], F32, tag="u2")
                    for e in range(8):
                        nc.tensor.matmul(pk[:],
                                         wk_t[e][:, fb * 128:(fb + 1) * 128],
                                         _xt(e),
                                         start=(e == 0), stop=(e == 7))
                    cols = slice(pb2 * 256, (pb2 + 1) * 256)
                    pl = sbtmp.tile([128, 512], F32R, tag="plain")
                    nc.scalar.copy(pl[:, :256], pk[:])
                    psw = psQ.tile([128, 256], F32, tag="s1")
                    nc.tensor.matmul(psw[:], pswap[:], pl[:, :256],
                                     start=True, stop=True)
                    t1 = sbtmp.tile([128, 512], F32, tag="ropeA")
                    nc.vector.tensor_mul(t1[:, :256], pk[:], kcos[:, cols])
                    t2 = sbtmp.tile([128, 512], F32, tag="ropeB")
                    nc.vector.tensor_mul(t2[:, :256], psw[:], ksin[:, cols])
                    nc.vector.tensor_add(
                        krot[:, fb * S + pb2 * 256:fb * S + (pb2 + 1) * 256],
                        t1[:, :256], t2[:, :256])
                # save q-slice chunks of xT (device q-slice is always rows 0:512)
                if pb2 < 2:
                    for e in range(8):
                        nc.vector.tensor_copy(
                            xtq[e][:, pb2 * 256:(pb2 + 1) * 256], _xt(e))


            for fb in range(8):
                qpass(fb)

            # ---- prefetch wo column-blocks into now-dead phase-1 slots
            wo_r = d_wo.ap().rearrange("(c p) f -> p c f", p=128)  # (128,8,1024)
            wo_tiles = []
            if phase_limit >= 3:
                for eb in range(8):
                    if eb < 4:
                        wo_c = sbxt2.tile([128, 1024], F32R, tag="xt4",
                                          name=f"wo{eb}")
                    elif eb < 6:
                        wo_c = sbstage.tile([128, 1024], F32R, tag="stage",
                                            name=f"wo{eb}")
                    else:
                        tg = "kcos" if eb == 6 else "ksin"
                        wo_c = sb1.tile([128, 1024], F32R, tag=tg,
                                        name=f"wo{eb}")
                    nc.sync.dma_start(
                        wo_c[:].rearrange("p (c f) -> p c f", c=8),
                        wo_r[:, :, eb * 128:(eb + 1) * 128])
                    wo_tiles.append(wo_c)

            # ---- attention, head-pipelined: head h's normalization tail is
            # emitted interleaved with head h+1's k-loop so the PE never
            # stalls on the serial tail chain.
            ofins = [None] * NH
            if phase_limit < 2:
                ysb0 = sbtmp.tile([128, 512], F32, tag="ysb")
                nc.vector.tensor_copy(ysb0[:], qrot[:, 0:512])
                nc.sync.dma_start(d_y[0:128, :], ysb0[:])
                return

            state = [dict() for _ in range(NH)]

            def kstep(h, k):
                st = state[h]
                kb = h % 2
                vh = h % 2
                if k == 0:
                    st["u1"] = psQ.tile([128, 512], F32, tag="u1",
                                        name=f"u1_{h}")
                    st["u2"] = psQ.tile([128, 512], F32, tag="u2",
                                        name=f"u2_{h}")
                    st["s1"] = psQ.tile([1, 512], F32, tag="s1",
                                        name=f"s1_{h}")
                    st["s2"] = psQ.tile([1, 512], F32, tag="s2",
                                        name=f"s2_{h}")
                    st["eps"] = [None] * 16
                if k < 16:
                    sp = psP.tile([128, 1024], F32, tag="sp",
                                  name=f"sp_{h}_{k}")
                    lo = kb * S + k * 128
                    nc.tensor.matmul(sp[:, 0:512],
                                     krot[0:64, lo:lo + 128],
                                     qrot[0:64, h * SQ:(h + 1) * SQ],
                                     start=True, stop=True)
                    nc.tensor.matmul(sp[:, 512:1024],
                                     krot[64:128, lo:lo + 128],
                                     qrot[64:128, h * SQ:(h + 1) * SQ],
                                     start=True, stop=True)
                    ep = sbe.tile([128, 1024], F32R, tag="epair",
                                  name=f"ep_{h}_{k}")
                    nc.scalar.activation(ep[:], sp[:], AF.Exp)
                    st["eps"][k] = ep
                if k >= 1:
                    j = k - 1
                    ep = st["eps"][j]
                    vsl = v_sb[j][:, vh * 128:vh * 128 + 128]
                    nc.tensor.matmul(st["u1"][:], vsl, ep[:, 0:512],
                                     start=(j == 0), stop=(j == 15))
                    nc.tensor.matmul(st["u2"][:], vsl, ep[:, 512:1024],
                                     start=(j == 0), stop=(j == 15))
                    nc.tensor.matmul(st["s1"][:], ones128[:], ep[:, 0:512],
                                     start=(j == 0), stop=(j == 15))
                    nc.tensor.matmul(st["s2"][:], ones128[:], ep[:, 512:1024],
                                     start=(j == 0), stop=(j == 15))

            def tail(h):
                st = state[h]
                sc1, sc2, sc3 = d_scr[h]
                r1 = smalls.tile([1, 512], F32, tag="r1", name=f"r1_{h}")
                r2 = smalls.tile([1, 512], F32, tag="r2", name=f"r2_{h}")
                nc.vector.reciprocal(r1[:], st["s1"][:])
                nc.vector.reciprocal(r2[:], st["s2"][:])
                r2l = smalls.tile([1, 512], F32, tag="r2l", name=f"r2l_{h}")
                nc.vector.tensor_scalar_mul(r2l[:], r2[:], lamsc[:])
                nc.sync.dma_start(sc1.ap(), r1[:].bitcast(F32))
                nc.sync.dma_start(sc2.ap(), r2l[:])
                bsb1 = sb1.tile([128, 512], F32, tag="qcos",
                                name=f"bsb1_{h}")
                bsb2 = sb1.tile([128, 512], F32, tag="qsin",
                                name=f"bsb2_{h}")
                nc.sync.dma_start(bsb1[:],
                                  sc1.ap().broadcast_to([128, 512]))
                nc.sync.dma_start(bsb2[:],
                                  sc2.ap().broadcast_to([128, 512]))
                # drain the U accumulators to SBUF right away so the psum
                # banks free for the next head (the DMA-bounce broadcast
                # above has multi-us latency)
                u1c = sbxtq.tile([128, 512], F32, tag="xtq", name=f"u1c_{h}")
                u2c = sbxtq.tile([128, 512], F32, tag="xtq", name=f"u2c_{h}")
                nc.vector.tensor_copy(u1c[:], st["u1"][:])
                nc.vector.tensor_copy(u2c[:], st["u2"][:])
                ta = sbxtq.tile([128, 512], F32, tag="xtq", name=f"ta_{h}")
                tb = sbxtq.tile([128, 512], F32, tag="xtq", name=f"tb_{h}")
                nc.vector.tensor_mul(ta[:], u1c[:], bsb1[:])
                nc.vector.tensor_mul(tb[:], u2c[:], bsb2[:])
                oh = sbxtq.tile([128, 512], F32R, tag="xtq", name=f"oh_{h}")
                nc.vector.tensor_sub(oh[:], ta[:], tb[:])
                sq = sbxtq.tile([128, 512], F32R, tag="xtq", name=f"sq_{h}")
                nc.vector.tensor_mul(sq[:], oh[:], oh[:])
                ssum = psQ.tile([1, 512], F32, tag="s1", name=f"ssum_{h}")
                nc.tensor.matmul(ssum[:], ones128[:], sq[:],
                                 start=True, stop=True)
                tln = smalls.tile([1, 512], F32, tag="rstd", name=f"tln_{h}")
                nc.scalar.activation(tln[:], ssum[:], AF.Ln,
                                     bias=epsb[:], scale=1.0 / 128.0)
                rinv = smalls.tile([1, 512], F32, tag="rinv",
                                   name=f"rinv_{h}")
                nc.scalar.activation(rinv[:], tln[:], AF.Exp,
                                     bias=zerob[:], scale=-0.5)
                nc.sync.dma_start(sc3.ap(), rinv[:])
                rrsb = sbxtq.tile([128, 512], F32, tag="xtq",
                                  name=f"rr_{h}")
                nc.sync.dma_start(rrsb[:],
                                  sc3.ap().broadcast_to([128, 512]))
                ofin = sbo.tile([128, 512], F32R, tag="o", name=f"ofin_{h}")
                nc.vector.tensor_mul(ofin[:], oh[:], rrsb[:])
                ofins[h] = ofin

            for hh in range(NH + 1):
                for k in range(17):
                    if hh < NH:
                        kstep(hh, k)
                    if hh >= 1 and k == 1:
                        tail(hh - 1)

            # ---- out-projection, streamed by output-feature block
            if phase_limit < 3:
                ysb0 = sbtmp.tile([128, 512], F32, tag="ysb")
                nc.vector.tensor_copy(ysb0[:], ofins[0][:])
                nc.sync.dma_start(d_y[0:128, :], ysb0[:])
                return
            for eb in range(8):
                wo_c = wo_tiles[eb]
                py = psP.tile([128, 512], F32, tag="sp")
                for hh in range(8):
                    nc.tensor.matmul(py[:], wo_c[:, hh * 128:(hh + 1) * 128],
                                     ofins[hh][:],
                                     start=(hh == 0), stop=(hh == 7))
                ysb = sbtmp.tile([128, 512], F32, tag="ysb")
                nc.vector.tensor_copy(ysb[:], py[:])
                nc.sync.dma_start(d_y[eb * 128:(eb + 1) * 128, :], ysb[:])

        if reps is None:
            _emit()
        else:
            with tc.For_i(0, reps, 1):
                _emit()

    nc.finalize()
    return nc


# -------------------- host side --------------------

def _make_runner(nc):
    import jax
    from jax.sharding import Mesh, PartitionSpec, NamedSharding
    from concourse import mybir
    from concourse.bass2jax import (_bass_exec_p, install_neuronx_cc_hook,
                                    partition_id_tensor)
    try:
        from jax.experimental.shard_map import shard_map
    except ImportError:
        from jax import shard_map

    install_neuronx_cc_hook()
    partition_name = nc.partition_id_tensor.name if nc.partition_id_tensor \
        else None
    in_names, out_names, out_avals = [], [], []
    for alloc in nc.m.functions[0].allocations:
        if not isinstance(alloc, mybir.MemoryLocationSet):
            continue
        name = alloc.memorylocations[0].name
        if alloc.kind == "ExternalInput":
            if name != partition_name:
                in_names.append(name)
        elif alloc.kind == "ExternalOutput":
            out_names.append(name)
            out_avals.append(jax.core.ShapedArray(
                tuple(alloc.tensor_shape), mybir.dt.np(alloc.dtype)))
    all_names = in_names + out_names + (
        [partition_name] if partition_name else [])

    def _body(*args):
        operands = list(args)
        if partition_name is not None:
            operands.append(partition_id_tensor())
        return tuple(_bass_exec_p.bind(
            *operands, out_avals=tuple(out_avals), in_names=tuple(all_names),
            out_names=tuple(out_names), lowering_input_output_aliases=(),
            sim_require_finite=True, sim_require_nnan=True, nc=nc))

    devices = jax.devices()[:NCORES]
    mesh = Mesh(np.asarray(devices), ("core",))
    n_params = len(in_names)
    n_outs = len(out_names)
    fn = jax.jit(
        shard_map(_body, mesh=mesh,
                  in_specs=(PartitionSpec("core"),) * (n_params + n_outs),
                  out_specs=(PartitionSpec("core"),) * n_outs,
                  check_rep=False),
        donate_argnums=tuple(range(n_params, n_params + n_outs)),
        keep_unused=True)
    sharding = NamedSharding(mesh, PartitionSpec("core"))
    return {
        "fn": fn, "in_names": in_names, "out_names": out_names,
        "out_avals": out_avals, "sharding": sharding, "jax": jax,
    }


def _prep_inputs(x, cos, sin, Wq, Wk, Wv, Wo, lambda_q1, lambda_k1,
                 lambda_q2, lambda_k2, sub_w):
    """Host-side prep: permutations, rope tables, per-core sharding."""
    x = np.asarray(x, np.float32)
    cos = np.asarray(cos, np.float32)
    sin = np.asarray(sin, np.float32)
    Wq = np.asarray(Wq, np.float32)
    Wk = np.asarray(Wk, np.float32)
    Wv = np.asarray(Wv, np.float32)
    Wo = np.asarray(Wo, np.float32)
    sub_w = np.asarray(sub_w, np.float32)

    lam1 = math.exp(float(np.sum(np.asarray(lambda_q1, np.float64)
                                 * np.asarray(lambda_k1, np.float64))))
    lam2 = math.exp(float(np.sum(np.asarray(lambda_q2, np.float64)
                                 * np.asarray(lambda_k2, np.float64))))
    lam = np.float32(lam1 - lam2 + LAMBDA_INIT)

    # de-interleave perm for head_dim 64 (j<32 -> 2j ; j>=32 -> 2(j-32)+1)
    perm = np.empty(HD, np.int64)
    perm[:32] = np.arange(32) * 2
    perm[32:] = np.arange(32) * 2 + 1
    scale = np.float32(HD ** -0.5)
    Wq_p = (Wq.reshape(E, 16, HD)[:, :, perm].reshape(E, E)
            * scale).astype(np.float32)
    Wk_p = Wk.reshape(E, 4, HD)[:, :, perm].reshape(E, 256).astype(np.float32)
    Wo_f = (Wo * np.tile(sub_w, NH)[:, None]).astype(np.float32)

    # rope tables in de-interleaved layout, (64,S) pattern tiled to 128
    cosT = cos.T  # (32, S)
    sinT = sin.T
    cos2 = np.tile(np.concatenate([cosT, cosT], 0), (2, 1))  # (128, S)
    sin2 = np.tile(np.concatenate([-sinT, sinT], 0), (2, 1))

    ones128 = np.ones((128, 1), np.float32)
    ones1x = np.ones((1, 128), np.float32)
    lam1x = np.full((1, 128), lam, np.float32)
    pswap = np.zeros((128, 128), np.float32)
    for d in range(128):
        base, off = (d // 64) * 64, d % 64
        pswap[base + (off + 32) % 64, d] = 1.0

    in_maps = []
    for c in range(NCORES):
        b, qs = divmod(c, 4)
        q0 = qs * SQ
        # rotate positions so this core's q-slice is rows [0,512); rotate
        # K rope tables identically (K/V permutation is softmax-invariant)
        xb = np.roll(x[b], -q0, axis=0)
        kcos_c = np.roll(cos2, -q0, axis=1)
        ksin_c = np.roll(sin2, -q0, axis=1)
        qcos_c = cos2[:, q0:q0 + SQ]
        qsin_c = sin2[:, q0:q0 + SQ]
        in_maps.append({
            "xb": np.ascontiguousarray(xb),
            "wq": Wq_p, "wk": Wk_p, "wv": Wv, "wo": Wo_f,
            "kcos": np.ascontiguousarray(kcos_c),
            "ksin": np.ascontiguousarray(ksin_c),
            "qcos": np.ascontiguousarray(qcos_c),
            "qsin": np.ascontiguousarray(qsin_c),
            "ones128": ones128, "ones1x": ones1x, "lam1x": lam1x,
            "pswap": pswap,
        })
    return in_maps


def _get_runner():
    if "runner" not in _CACHE:
        nc = _build()
        _CACHE["runner"] = _make_runner(nc)
    return _CACHE["runner"]


def _stage(runner, in_maps):
    jax = runner["jax"]
    concat = [np.concatenate([np.asarray(m[n]) for m in in_maps], axis=0)
              for n in runner["in_names"]]
    return [jax.device_put(a, runner["sharding"]) for a in concat]


def _zeros(runner):
    jax = runner["jax"]
    return [jax.device_put(
        np.zeros((NCORES * av.shape[0], *av.shape[1:]), av.dtype),
        runner["sharding"]) for av in runner["out_avals"]]


def _execute(runner, ins_dev):
    jax = runner["jax"]
    outs = runner["fn"](*ins_dev, *_zeros(runner))
    jax.block_until_ready(outs)
    return outs


def _gather(runner, outs):
    av = runner["out_avals"][0]
    yT_all = np.asarray(outs[0]).reshape(NCORES, *av.shape)
    y = np.empty((B, S, E), np.float32)
    for c in range(NCORES):
        b, qs = divmod(c, 4)
        y[b, qs * SQ:(qs + 1) * SQ, :] = yT_all[c].T
    return y


def kernel(**inputs) -> np.ndarray:
    runner = _get_runner()
    in_maps = _prep_inputs(**inputs)
    ins_dev = _stage(runner, in_maps)
    outs = _execute(runner, ins_dev)
    return _gather(runner, outs)



# revision 2
# speedup vs baseline: 1.1366x; 1.1366x over previous
"""DifferentialAttention TRN2 Bass kernel — 8-core SPMD, bf16 compute.

Sharding: core c handles batch b = c//4, query rows [512*(c%4), 512*(c%4+1)).
Each core computes full K/V for its batch (replicated across the 4 cores of
that batch), its 512-query slice of Q, attention, subnorm, and the full
out-projection for its query slice. No collectives; host concatenates.

Per-core dataflow (feature-major):
  x_b --DMA--> stage --PE transpose (f32r)--> xT --copy--> bf16
  K^T = Wk'.T @ xT ; V = xT.T @ Wv ; Q^T = Wq'.T @ xT[:, qslice]   (bf16)
  RoPE (de-interleaved layout; pair-swap via pswap matmul; DVE mul/add)
  scores^T[k,q] per comp pair: two K=64 matmuls row-tiled at (0,0)/(64,0)
  E = exp(scores^T) on ACT (psum -> bf16 sbuf)
  U_i = V.T-contract E_i (PE, K=128 accumulate over 16 blocks)
  s_i = ones.T @ E_i, col-tiled into strips {0,32} of one psum bank
  1/s via DVE reciprocal; [128,512] broadcasts via K=1 ones-matmul (PSUM)
  O_h = U1*B1 - U2*B2 ; mean(O^2) via ones-matmul into strip 64
  ssum batched over heads in [1,4096]; one Ln + one Exp for all rstd
  y^T = Wo'.T @ (O_h * rstd_h) accumulated over heads; DMA y^T out.

Weights host-permuted: de-interleaved RoPE pairs, attention scale folded
into Wq, subnorm weight folded into Wo rows; all weights/tables in bf16.
"""
import sys

sys.path.insert(0, '/opt/trn_rl_repo')

import math
from contextlib import ExitStack

import numpy as np

B, S, E = 2, 2048, 1024
NH, HD = 8, 64
SQ = 512            # queries per core
NCORES = 8
LAMBDA_INIT = 0.8
EPS = 1e-5

_CACHE = {}


def _build(reps=None, phase_limit=3):
    import concourse.bacc as bacc
    import concourse.tile as tile
    from concourse import mybir
    from concourse.masks import make_identity

    dt = mybir.dt
    F32, F32R, BF16 = dt.float32, dt.float32r, dt.bfloat16
    AF = mybir.ActivationFunctionType

    nc = bacc.Bacc("TRN2", num_devices=NCORES)
    d_x = nc.declare_dram_parameter("xb", [S, E], F32, isOutput=False)
    d_wq = nc.declare_dram_parameter("wq", [E, E], BF16, isOutput=False)
    d_wk = nc.declare_dram_parameter("wk", [E, 256], BF16, isOutput=False)
    d_wv = nc.declare_dram_parameter("wv", [E, 256], BF16, isOutput=False)
    d_wo = nc.declare_dram_parameter("wo", [E, E], BF16, isOutput=False)
    d_kcos = nc.declare_dram_parameter("kcos", [128, S], BF16, isOutput=False)
    d_ksin = nc.declare_dram_parameter("ksin", [128, S], BF16, isOutput=False)
    d_qcos = nc.declare_dram_parameter("qcos", [128, SQ], BF16, isOutput=False)
    d_qsin = nc.declare_dram_parameter("qsin", [128, SQ], BF16, isOutput=False)
    d_ones128 = nc.declare_dram_parameter("ones128", [128, 1], BF16,
                                          isOutput=False)
    d_ones1x = nc.declare_dram_parameter("ones1x", [1, 128], BF16,
                                         isOutput=False)
    d_lam1x = nc.declare_dram_parameter("lam1x", [1, 128], BF16,
                                        isOutput=False)
    d_pswap = nc.declare_dram_parameter("pswap", [128, 128], BF16,
                                        isOutput=False)
    d_y = nc.declare_dram_parameter("yT", [E, SQ], F32, isOutput=True)

    with tile.TileContext(nc) as tc, ExitStack() as ctx:
        sb1 = ctx.enter_context(tc.tile_pool(name="sb1", bufs=1))
        sbstage = ctx.enter_context(tc.tile_pool(name="sbstage", bufs=2))
        sbxt2 = ctx.enter_context(tc.tile_pool(name="sbxt2", bufs=4))
        sbxtq = ctx.enter_context(tc.tile_pool(name="sbxtq", bufs=8))
        sbw = ctx.enter_context(tc.tile_pool(name="sbw", bufs=2))
        sbkv = ctx.enter_context(tc.tile_pool(name="sbkv", bufs=8))
        sbv = ctx.enter_context(tc.tile_pool(name="sbv", bufs=16))
        sbe = ctx.enter_context(tc.tile_pool(name="sbe", bufs=7))
        sbo = ctx.enter_context(tc.tile_pool(name="sbo", bufs=8))
        sbof = ctx.enter_context(tc.tile_pool(name="sbof", bufs=8))
        sbwo = ctx.enter_context(tc.tile_pool(name="sbwo", bufs=8))
        sbtl = ctx.enter_context(tc.tile_pool(name="sbtl", bufs=4))
        sbtmp = ctx.enter_context(tc.tile_pool(name="sbtmp", bufs=2))
        smalls = ctx.enter_context(tc.tile_pool(name="smalls", bufs=2))
        psP = ctx.enter_context(tc.tile_pool(name="psP", bufs=2, space="PSUM"))
        psQ = ctx.enter_context(tc.tile_pool(name="psQ", bufs=1, space="PSUM"))
        psS = ctx.enter_context(tc.tile_pool(name="psS", bufs=2, space="PSUM"))

        def _emit():
            # ---- stage-in of the first x group goes first so its DMA is not
            # queued behind several MB of weight/table loads.
            def load_stage(pb2):
                stg = sbstage.tile([128, 2048], F32, tag="stage",
                                   name=f"stg{pb2}")
                for h2 in range(2):
                    pb = pb2 * 2 + h2
                    nc.sync.dma_start(stg[:, h2 * 1024:(h2 + 1) * 1024],
                                      d_x[pb * 128:(pb + 1) * 128, :])
                return stg

            stg_next = load_stage(0)

            # ---- persistent tiles
            krot = sb1.tile([128, 2 * S], BF16, tag="krot")
            qrot = sb1.tile([128, 8 * SQ], BF16, tag="qrot")
            kcos = sb1.tile([128, S], BF16, tag="kcos")
            ksin = sb1.tile([128, S], BF16, tag="ksin")
            qcos = sb1.tile([128, SQ], BF16, tag="qcos")
            qsin = sb1.tile([128, SQ], BF16, tag="qsin")
            ident = sb1.tile([128, 128], F32, tag="ident")
            ones128 = sb1.tile([128, 1], BF16, tag="ones128")
            ones1x = sb1.tile([1, 128], BF16, tag="ones1x")
            lam1x = sb1.tile([1, 128], BF16, tag="lam1x")
            pswap = sb1.tile([128, 128], BF16, tag="pswap")
            ssum8 = sb1.tile([1, 8 * SQ], BF16, tag="ssum8")
            tln = sb1.tile([1, 8 * SQ], BF16, tag="tln")
            rstd = sb1.tile([1, 8 * SQ], BF16, tag="rstd")
            epsb = sb1.tile([1, 1], F32, tag="epsb")
            nc.vector.memset(epsb[:], EPS)
            zerob = sb1.tile([1, 1], F32, tag="zerob")
            nc.vector.memset(zerob[:], 0.0)

            nc.sync.dma_start(kcos[:], d_kcos[:])
            nc.sync.dma_start(ksin[:], d_ksin[:])
            nc.sync.dma_start(qcos[:], d_qcos[:])
            nc.sync.dma_start(qsin[:], d_qsin[:])
            nc.sync.dma_start(ones128[:], d_ones128[:])
            nc.sync.dma_start(ones1x[:], d_ones1x[:])
            nc.sync.dma_start(lam1x[:], d_lam1x[:])
            nc.sync.dma_start(pswap[:], d_pswap[:])
            make_identity(nc, ident[:])

            # wk/wv resident in full (8 e-chunks each)
            wk_t = [sbkv.tile([128, 256], BF16, tag="wk", name=f"wk{e}")
                    for e in range(8)]
            wv_t = [sbkv.tile([128, 256], BF16, tag="wv", name=f"wv{e}")
                    for e in range(8)]
            for e in range(8):
                nc.sync.dma_start(wk_t[e][:], d_wk[e * 128:(e + 1) * 128, :])
                nc.sync.dma_start(wv_t[e][:], d_wv[e * 128:(e + 1) * 128, :])

            v_sb = [sbv.tile([128, 256], BF16, tag="v", name=f"v{p}")
                    for p in range(16)]
            xtq = [sbxtq.tile([128, SQ], BF16, tag="xtq", name=f"xtq{e}")
                   for e in range(8)]

            # wo column-blocks, needed only at the very end
            wo_r = d_wo.ap().rearrange("(c p) f -> p c f", p=128)
            wo_tiles = []
            for eb in range(8):
                wo_c = sbwo.tile([128, 1024], BF16, tag="wo", name=f"wo{eb}")
                nc.sync.dma_start(
                    wo_c[:].rearrange("p (c f) -> p c f", c=8),
                    wo_r[:, :, eb * 128:(eb + 1) * 128])
                wo_tiles.append(wo_c)

            # ---- phase 0+1: transpose x by 256-position groups; K^T, V proj.
            # One NEFF serves all 8 cores: the host pre-rotates x along S so
            # every core's q-slice sits at rows [0, 512). Jointly permuting
            # keys and values leaves softmax attention invariant; the K rope
            # tables are rotated identically so K keeps original positions.
            for pb2 in range(8):
                stg = stg_next
                if pb2 < 7:
                    stg_next = load_stage(pb2 + 1)
                xt4 = [sbxt2.tile([128, 1024], BF16, tag="xt4",
                                  name=f"xt4_{pb2}_{eh}") for eh in range(2)]

                def _xt(e):
                    return xt4[e // 4][:, (e % 4) * 256:(e % 4) * 256 + 256]

                for eh in range(2):  # split 8 e-chunks over 2 psum groups
                    pt = psP.tile([128, 1024], F32, tag="sp")
                    for j in range(4):
                        e = eh * 4 + j
                        for h2 in range(2):
                            nc.tensor.transpose(
                                pt[:, j * 256 + h2 * 128:
                                   j * 256 + h2 * 128 + 128].bitcast(F32R),
                                stg[:, h2 * 1024 + e * 128:
                                    h2 * 1024 + e * 128 + 128].bitcast(F32R),
                                ident[:].bitcast(F32R))
                    nc.vector.tensor_copy(xt4[eh][:], pt[:])
                # V for the two position blocks of this group
                for h2 in range(2):
                    pb = pb2 * 2 + h2
                    pv = psQ.tile([128, 256], F32, tag="u1")
                    for e in range(8):
                        xe = _xt(e)
                        nc.tensor.matmul(pv[:], xe[:, h2 * 128:h2 * 128 + 128],
                                         wv_t[e][:],
                                         start=(e == 0), stop=(e == 7))
                    nc.vector.tensor_copy(v_sb[pb][:], pv[:])
                # K^T columns for this group, + rope
                for fb in range(2):
                    pk = psQ.tile([128, 256], F32, tag="u2")
                    for e in range(8):
                        nc.tensor.matmul(pk[:],
                                         wk_t[e][:, fb * 128:(fb + 1) * 128],
                                         _xt(e),
                                         start=(e == 0), stop=(e == 7))
                    cols = slice(pb2 * 256, (pb2 + 1) * 256)
                    pl = sbtmp.tile([128, 256], BF16, tag="plain")
                    nc.vector.tensor_copy(pl[:], pk[:])
                    psw = psS.tile([128, 256], F32, tag="s12")
                    nc.tensor.matmul(psw[:], pswap[:], pl[:],
                                     start=True, stop=True)
                    t1 = sbtmp.tile([128, 256], F32, tag="ropeA")
                    nc.vector.tensor_mul(t1[:], pk[:], kcos[:, cols])
                    t2 = sbtmp.tile([128, 256], F32, tag="ropeB")
                    nc.vector.tensor_mul(t2[:], psw[:], ksin[:, cols])
                    nc.vector.tensor_add(
                        krot[:, fb * S + pb2 * 256:fb * S + (pb2 + 1) * 256],
                        t1[:], t2[:])
                # save q-slice chunks of xT (device q-slice is rows 0:512)
                if pb2 < 2:
                    for e in range(8):
                        nc.vector.tensor_copy(
                            xtq[e][:, pb2 * 256:(pb2 + 1) * 256], _xt(e))

            # ---- Q projection + rope, per 128-feature block (head pair)
            wq_r = d_wq.ap().rearrange("(c p) f -> p c f", p=128)
            for fb in range(8):
                wq_c = sbw.tile([128, 1024], BF16, tag="wcol",
                                name=f"wq{fb}")
                nc.sync.dma_start(
                    wq_c[:].rearrange("p (c f) -> p c f", c=8),
                    wq_r[:, :, fb * 128:(fb + 1) * 128])
                pq = psQ.tile([128, 512], F32, tag="u1" if fb % 2 == 0
                              else "u2", name=f"pq{fb}")
                for e in range(8):
                    nc.tensor.matmul(pq[:], wq_c[:, e * 128:(e + 1) * 128],
                                     xtq[e][:],
                                     start=(e == 0), stop=(e == 7))
                pl = sbtmp.tile([128, 512], BF16, tag="plain",
                                name=f"qpl{fb}")
                nc.vector.tensor_copy(pl[:], pq[:])
                psw = psS.tile([128, 512], F32, tag="s12", name=f"qsw{fb}")
                nc.tensor.matmul(psw[:], pswap[:], pl[:],
                                 start=True, stop=True)
                t1 = sbtmp.tile([128, 512], F32, tag="ropeA",
                                name=f"qt1_{fb}")
                nc.vector.tensor_mul(t1[:], pq[:], qcos[:])
                t2 = sbtmp.tile([128, 512], F32, tag="ropeB",
                                name=f"qt2_{fb}")
                nc.vector.tensor_mul(t2[:], psw[:], qsin[:])
                nc.vector.tensor_add(qrot[:, fb * SQ:(fb + 1) * SQ],
                                     t1[:], t2[:])

            if phase_limit < 2:
                ysb0 = sbtmp.tile([128, 512], F32, tag="ysb")
                nc.vector.tensor_copy(ysb0[:],
                                      qrot[:, 0:512].bitcast(F32)[:, 0:512])
                nc.sync.dma_start(d_y[0:128, :], ysb0[:])
                return

            # ---- attention, head-pipelined
            ohs = [None] * NH
            state = [dict() for _ in range(NH)]

            def kstep(h, k):
                st = state[h]
                kb = h % 2
                vh = h % 2
                if k == 0:
                    st["u1"] = psQ.tile([128, 512], F32, tag="u1",
                                        name=f"u1_{h}")
                    st["u2"] = psQ.tile([128, 512], F32, tag="u2",
                                        name=f"u2_{h}")
                    st["s12"] = psS.tile([128, 512], F32, tag="s12",
                                         name=f"s12_{h}")
                    st["eps"] = [None] * 16
                if k < 16:
                    sp = psP.tile([128, 1024], F32, tag="sp",
                                  name=f"sp_{h}_{k}")
                    lo = kb * S + k * 128
                    nc.tensor.matmul(sp[:, 0:512],
                                     krot[0:64, lo:lo + 128],
                                     qrot[0:64, h * SQ:(h + 1) * SQ],
                                     start=True, stop=True)
                    nc.tensor.matmul(sp[:, 512:1024],
                                     krot[64:128, lo:lo + 128],
                                     qrot[64:128, h * SQ:(h + 1) * SQ],
                                     start=True, stop=True)
                    ep = sbe.tile([128, 1024], BF16, tag="epair",
                                  name=f"ep_{h}_{k}")
                    nc.scalar.activation(ep[:], sp[:], AF.Exp)
                    st["eps"][k] = ep
                if k >= 1:
                    j = k - 1
                    ep = st["eps"][j]
                    vsl = v_sb[j][:, vh * 128:vh * 128 + 128]
                    nc.tensor.matmul(st["u1"][:], vsl, ep[:, 0:512],
                                     start=(j == 0), stop=(j == 15))
                    nc.tensor.matmul(st["u2"][:], vsl, ep[:, 512:1024],
                                     start=(j == 0), stop=(j == 15))
                    nc.tensor.matmul(st["s12"][0:1, :], ones128[:],
                                     ep[:, 0:512],
                                     start=(j == 0), stop=(j == 15),
                                     tile_position=(0, 0))
                    nc.tensor.matmul(st["s12"][32:33, :], ones128[:],
                                     ep[:, 512:1024],
                                     start=(j == 0), stop=(j == 15),
                                     tile_position=(0, 32))

            def tail(h):
                st = state[h]
                s12 = st["s12"]
                r1 = smalls.tile([1, 512], BF16, tag="r1", name=f"r1_{h}")
                r2 = smalls.tile([1, 512], BF16, tag="r2", name=f"r2_{h}")
                with nc.allow_low_precision("bf16 softmax denominators"):
                    nc.vector.reciprocal(r1[:], s12[0:1, :])
                    nc.vector.reciprocal(r2[:], s12[32:33, :])
                bb = psP.tile([128, 1024], F32, tag="sp", name=f"bb_{h}")
                nc.tensor.matmul(bb[:, 0:512], ones1x[:], r1[:],
                                 start=True, stop=True)
                nc.tensor.matmul(bb[:, 512:1024], lam1x[:], r2[:],
                                 start=True, stop=True)
                ta = sbtl.tile([128, 512], F32, tag="ta", name=f"ta_{h}")
                tb = sbtl.tile([128, 512], F32, tag="tb", name=f"tb_{h}")
                nc.vector.tensor_mul(ta[:], st["u1"][:], bb[:, 0:512])
                nc.vector.tensor_mul(tb[:], st["u2"][:], bb[:, 512:1024])
                oh = sbo.tile([128, 512], BF16, tag="o", name=f"oh_{h}")
                nc.vector.tensor_sub(oh[:], ta[:], tb[:])
                sq = sbtl.tile([128, 512], BF16, tag="sq", name=f"sq_{h}")
                nc.vector.tensor_mul(sq[:], oh[:], oh[:])
                nc.tensor.matmul(s12[64:65, :], ones128[:], sq[:],
                                 start=True, stop=True, tile_position=(0, 64))
                with nc.allow_low_precision("bf16 rms sums"):
                    nc.vector.tensor_copy(ssum8[0:1, h * 512:(h + 1) * 512],
                                          s12[64:65, :])
                ohs[h] = oh

            for hh in range(NH + 1):
                if hh >= 1:
                    tail(hh - 1)
                for k in range(17):
                    if hh < NH:
                        kstep(hh, k)

            # ---- batched RMS rstd for all heads: one Ln + one Exp
            nc.scalar.activation(tln[:], ssum8[:], AF.Ln,
                                 bias=epsb[:], scale=1.0 / 128.0)
            nc.scalar.activation(rstd[:], tln[:], AF.Exp,
                                 bias=zerob[:], scale=-0.5)

            # rstd broadcasts + final per-head scaling
            ofins = [None] * NH
            for hp in range(4):
                rb = psP.tile([128, 1024], F32, tag="sp", name=f"rb_{hp}")
                for g in range(2):
                    h = 2 * hp + g
                    nc.tensor.matmul(rb[:, g * 512:(g + 1) * 512], ones1x[:],
                                     rstd[0:1, h * 512:(h + 1) * 512],
                                     start=True, stop=True)
                for g in range(2):
                    h = 2 * hp + g
                    ofin = sbof.tile([128, 512], BF16, tag="of",
                                     name=f"of_{h}")
                    nc.vector.tensor_mul(ofin[:], ohs[h][:],
                                         rb[:, g * 512:(g + 1) * 512])
                    ofins[h] = ofin

            # ---- out-projection, streamed by output-feature block
            if phase_limit < 3:
                ysb0 = sbtmp.tile([128, 512], F32, tag="ysb")
                nc.vector.tensor_copy(ysb0[:], ofins[0][:])
                nc.sync.dma_start(d_y[0:128, :], ysb0[:])
                return
            for eb in range(8):
                wo_c = wo_tiles[eb]
                py = psP.tile([128, 1024], F32, tag="sp", name=f"py{eb}")
                for hh in range(8):
                    nc.tensor.matmul(py[:, 0:512],
                                     wo_c[:, hh * 128:(hh + 1) * 128],
                                     ofins[hh][:],
                                     start=(hh == 0), stop=(hh == 7))
                ysb = sbtmp.tile([128, 512], F32, tag="ysb")
                nc.vector.tensor_copy(ysb[:], py[:, 0:512])
                nc.sync.dma_start(d_y[eb * 128:(eb + 1) * 128, :], ysb[:])

        if reps is None:
            _emit()
        else:
            with tc.For_i(0, reps, 1):
                _emit()

    nc.finalize()
    return nc


# -------------------- host side --------------------

def _make_runner(nc):
    import jax
    from jax.sharding import Mesh, PartitionSpec, NamedSharding
    from concourse import mybir
    from concourse.bass2jax import (_bass_exec_p, install_neuronx_cc_hook,
                                    partition_id_tensor)
    try:
        from jax.experimental.shard_map import shard_map
    except ImportError:
        from jax import shard_map

    install_neuronx_cc_hook()
    partition_name = nc.partition_id_tensor.name if nc.partition_id_tensor \
        else None
    in_names, out_names, out_avals = [], [], []
    for alloc in nc.m.functions[0].allocations:
        if not isinstance(alloc, mybir.MemoryLocationSet):
            continue
        name = alloc.memorylocations[0].name
        if alloc.kind == "ExternalInput":
            if name != partition_name:
                in_names.append(name)
        elif alloc.kind == "ExternalOutput":
            out_names.append(name)
            out_avals.append(jax.core.ShapedArray(
                tuple(alloc.tensor_shape), mybir.dt.np(alloc.dtype)))
    all_names = in_names + out_names + (
        [partition_name] if partition_name else [])

    def _body(*args):
        operands = list(args)
        if partition_name is not None:
            operands.append(partition_id_tensor())
        return tuple(_bass_exec_p.bind(
            *operands, out_avals=tuple(out_avals), in_names=tuple(all_names),
            out_names=tuple(out_names), lowering_input_output_aliases=(),
            sim_require_finite=True, sim_require_nnan=True, nc=nc))

    devices = jax.devices()[:NCORES]
    mesh = Mesh(np.asarray(devices), ("core",))
    n_params = len(in_names)
    n_outs = len(out_names)
    fn = jax.jit(
        shard_map(_body, mesh=mesh,
                  in_specs=(PartitionSpec("core"),) * (n_params + n_outs),
                  out_specs=(PartitionSpec("core"),) * n_outs,
                  check_rep=False),
        donate_argnums=tuple(range(n_params, n_params + n_outs)),
        keep_unused=True)
    sharding = NamedSharding(mesh, PartitionSpec("core"))
    return {
        "fn": fn, "in_names": in_names, "out_names": out_names,
        "out_avals": out_avals, "sharding": sharding, "jax": jax,
    }


def _prep_inputs(x, cos, sin, Wq, Wk, Wv, Wo, lambda_q1, lambda_k1,
                 lambda_q2, lambda_k2, sub_w):
    """Host-side prep: permutations, rope tables, per-core sharding."""
    import ml_dtypes
    bf16 = ml_dtypes.bfloat16

    x = np.asarray(x, np.float32)
    cos = np.asarray(cos, np.float32)
    sin = np.asarray(sin, np.float32)
    Wq = np.asarray(Wq, np.float32)
    Wk = np.asarray(Wk, np.float32)
    Wv = np.asarray(Wv, np.float32)
    Wo = np.asarray(Wo, np.float32)
    sub_w = np.asarray(sub_w, np.float32)

    lam1 = math.exp(float(np.sum(np.asarray(lambda_q1, np.float64)
                                 * np.asarray(lambda_k1, np.float64))))
    lam2 = math.exp(float(np.sum(np.asarray(lambda_q2, np.float64)
                                 * np.asarray(lambda_k2, np.float64))))
    lam = np.float32(lam1 - lam2 + LAMBDA_INIT)

    # de-interleave perm for head_dim 64 (j<32 -> 2j ; j>=32 -> 2(j-32)+1)
    perm = np.empty(HD, np.int64)
    perm[:32] = np.arange(32) * 2
    perm[32:] = np.arange(32) * 2 + 1
    scale = np.float32(HD ** -0.5)
    Wq_p = (Wq.reshape(E, 16, HD)[:, :, perm].reshape(E, E)
            * scale).astype(bf16)
    Wk_p = Wk.reshape(E, 4, HD)[:, :, perm].reshape(E, 256).astype(bf16)
    Wv_p = Wv.astype(bf16)
    Wo_f = (Wo * np.tile(sub_w, NH)[:, None]).astype(bf16)

    # rope tables in de-interleaved layout, (64,S) pattern tiled to 128
    cosT = cos.T  # (32, S)
    sinT = sin.T
    cos2 = np.tile(np.concatenate([cosT, cosT], 0), (2, 1))  # (128, S)
    sin2 = np.tile(np.concatenate([-sinT, sinT], 0), (2, 1))

    ones128 = np.ones((128, 1), bf16)
    ones1x = np.ones((1, 128), bf16)
    lam1x = np.full((1, 128), lam, bf16)
    pswap = np.zeros((128, 128), np.float32)
    for d in range(128):
        base, off = (d // 64) * 64, d % 64
        pswap[base + (off + 32) % 64, d] = 1.0
    pswap = pswap.astype(bf16)

    in_maps = []
    for c in range(NCORES):
        b, qs = divmod(c, 4)
        q0 = qs * SQ
        # rotate positions so this core's q-slice is rows [0,512); rotate
        # K rope tables identically (K/V permutation is softmax-invariant)
        xb = np.roll(x[b], -q0, axis=0)
        kcos_c = np.roll(cos2, -q0, axis=1)
        ksin_c = np.roll(sin2, -q0, axis=1)
        qcos_c = cos2[:, q0:q0 + SQ]
        qsin_c = sin2[:, q0:q0 + SQ]
        in_maps.append({
            "xb": np.ascontiguousarray(xb),
            "wq": Wq_p, "wk": Wk_p, "wv": Wv_p, "wo": Wo_f,
            "kcos": np.ascontiguousarray(kcos_c.astype(bf16)),
            "ksin": np.ascontiguousarray(ksin_c.astype(bf16)),
            "qcos": np.ascontiguousarray(qcos_c.astype(bf16)),
            "qsin": np.ascontiguousarray(qsin_c.astype(bf16)),
            "ones128": ones128, "ones1x": ones1x, "lam1x": lam1x,
            "pswap": pswap,
        })
    return in_maps


def _get_runner():
    if "runner" not in _CACHE:
        nc = _build()
        _CACHE["runner"] = _make_runner(nc)
    return _CACHE["runner"]


def _stage(runner, in_maps):
    jax = runner["jax"]
    concat = [np.concatenate([np.asarray(m[n]) for m in in_maps], axis=0)
              for n in runner["in_names"]]
    return [jax.device_put(a, runner["sharding"]) for a in concat]


def _zeros(runner):
    jax = runner["jax"]
    return [jax.device_put(
        np.zeros((NCORES * av.shape[0], *av.shape[1:]), av.dtype),
        runner["sharding"]) for av in runner["out_avals"]]


def _execute(runner, ins_dev):
    jax = runner["jax"]
    outs = runner["fn"](*ins_dev, *_zeros(runner))
    jax.block_until_ready(outs)
    return outs


def _gather(runner, outs):
    av = runner["out_avals"][0]
    yT_all = np.asarray(outs[0]).reshape(NCORES, *av.shape)
    y = np.empty((B, S, E), np.float32)
    for c in range(NCORES):
        b, qs = divmod(c, 4)
        y[b, qs * SQ:(qs + 1) * SQ, :] = yT_all[c].T
    return y


def kernel(**inputs) -> np.ndarray:
    runner = _get_runner()
    in_maps = _prep_inputs(**inputs)
    ins_dev = _stage(runner, in_maps)
    outs = _execute(runner, ins_dev)
    return _gather(runner, outs)
